# revision 8
# baseline (speedup 1.0000x reference)
"""Deformable Conv2d Lite (K=3) on 8 Trainium2 NeuronCores.

Sharding: data-parallel over batch x image-half. Core n handles sample n//2,
image rows [64*(n%2), 64*(n%2)+64). Weight replicated.

Device pipeline per core (~510us HW, rel err ~4e-4):
  1. DVE: from raw offsets compute, per (tap, pixel), a gather index into a
     row-pair-interleaved fp16 NHWC layout of x (row r of xpair holds image
     rows r, r+1 at one column; idx = clamp(y0)*128 + clamp(x0)), plus 4
     bilinear corner weights (eq-masked so clamping/out-of-image reproduces
     zero-padding exactly; floor built from int-cast + is_gt since mod is
     not a valid HW ALU op).
  2. SWDGE dma_gather (fp16): one 512B descriptor per (tap, pixel) fetches
     the full 2x2 x 64ch bilinear patch from DRAM; 108 calls of <=768
     indices (descriptor-ring capacity) round-robined over 4 SWDGE queues.
  3. DVE: single in-place weight multiply (weights broadcast along channels
     via stride-0 AP). The four corner ADDITIONS are folded into the conv
     matmul contraction instead of running on DVE (Pool descriptor-gen and
     DVE 2-port ops serialize on a shared SBUF port, so DVE work directly
     lengthens the gather phase).
  4. PE: fp16 transposes of the weighted patches to channel-major
     (K = 2 corners x 64ch = 128 per tap-half), then conv matmuls with
     per-tap weight slabs (W_t.T stacked twice) accumulating f32 in PSUM
     over all 9 taps x 2 halves.
  5. ACT: PSUM->SBUF copies of transposed tiles, and final bias-add on the
     conv PSUM; DMA out.
"""

import sys

for _p in ("/opt/trn_rl_repo",):
    if _p not in sys.path:
        sys.path.insert(0, _p)

import numpy as np

import concourse.bass as bass
import concourse.tile as tile
from concourse import bacc, mybir
from concourse.bass_utils import run_bass_kernel_spmd

F32 = mybir.dt.float32
F16 = mybir.dt.float16
I16 = mybir.dt.int16
Alu = mybir.AluOpType
Act = mybir.ActivationFunctionType

B, C, H, W = 4, 64, 128, 128
OC, KK = 64, 9
HALF = H // 2            # rows per core
PIX = HALF * W           # 8192 pixels per core
NCHUNK = 4
CPIX = PIX // NCHUNK     # 2048 pixels per chunk
CSLOT = CPIX // 128      # 16 slots per chunk
SLOTS = PIX // 128       # 64
NPAIR = 5                # ceil(9/2) tap pairs

XROWS = 2 * PIX + 1      # interleaved pair-row count incl. pad


import os


def build_program(loop_n: int = 0, ablate: str = ""):
    """Build the per-core Bass program. loop_n>0 wraps the body in a device
    For_i loop (for wall-clock timing); loop_n==0 emits the plain body.
    ablate: comma-set of {nogather, nodve, nope} for perf bisection."""
    abl = set(ablate.split(",")) if ablate else set()
    import os
    nc = bacc.Bacc("TRN2", target_bir_lowering=False, debug=False, num_devices=8,
                   num_swdge_queues=4,
                   dynamic_dma_scratch_size=int(os.environ.get("DDS", "16384")))

    xp = nc.dram_tensor("xpair", [XROWS, 128], F16, kind="ExternalInput").ap()
    offs = nc.dram_tensor("offs", [PIX, 2 * KK], F32, kind="ExternalInput").ap()
    wp = nc.dram_tensor("wpair", [128, KK * OC], F16, kind="ExternalInput").ap()
    yyd = nc.dram_tensor("yy", [128, SLOTS], F32, kind="ExternalInput").ap()
    xxd = nc.dram_tensor("xx", [128, 1], F32, kind="ExternalInput").ap()
    idd = nc.dram_tensor("ident", [128, 128], F16, kind="ExternalInput").ap()
    bsd = nc.dram_tensor("bias", [OC, 1], F32, kind="ExternalInput").ap()
    out = nc.dram_tensor("out", [OC, PIX], F32, kind="ExternalOutput").ap()

    with tile.TileContext(nc) as tc:
        import contextlib

        with contextlib.ExitStack() as ctx:
            cpool = ctx.enter_context(tc.tile_pool(name="consts", bufs=1))
            apool = ctx.enter_context(tc.tile_pool(name="stageA", bufs=1))
            gpool = ctx.enter_context(tc.tile_pool(name="gather", bufs=int(os.environ.get("GBUFS", "4"))))
            stpool = ctx.enter_context(tc.tile_pool(name="stmaj", bufs=int(os.environ.get("SBUFS", "5"))))
            opool = ctx.enter_context(tc.tile_pool(name="outsb", bufs=2))
            ptpool = ctx.enter_context(
                tc.tile_pool(name="psumT", bufs=2, space="PSUM")
            )
            pcpool = ctx.enter_context(
                tc.tile_pool(name="psumC", bufs=1, space="PSUM")
            )

            # ---- constants -------------------------------------------------
            xx = cpool.tile([128, 1], F32)
            nc.sync.dma_start(xx[:], xxd[:, :])
            yy = cpool.tile([128, SLOTS], F32)
            nc.sync.dma_start(yy[:], yyd[:, :])
            ident = cpool.tile([128, 128], F16)
            nc.sync.dma_start(ident[:], idd[:, :])
            wpt = cpool.tile([128, KK * OC], F16)
            nc.sync.dma_start(wpt[:], wp[:, :])
            bias = cpool.tile([OC, 1], F32)
            nc.sync.dma_start(bias[:], bsd[:, :])

            def body(_iv=None):
                if "gonly" in abl:
                    idxw = apool.tile([128, KK, SLOTS * 8], I16, name="idxw")
                    nc.gpsimd.iota(
                        idxw[:].rearrange("p a b -> p (a b)"),
                        pattern=[[3, KK * SLOTS * 8]],
                        base=0,
                        channel_multiplier=0,
                    )
                    xsrc = bass.AP(xp.tensor, 0, [[128, 2 * PIX], [1, 256]])
                    regs = {6: nc.gpsimd.to_reg(768), 4: nc.gpsimd.to_reg(512)}
                    gi = 0
                    for ch in range(NCHUNK):
                        for t in range(KK):
                            g = gpool.tile([128, CSLOT, 4, 64], F16, name="g")
                            for s0, ns in ((0, 6), (6, 6), (12, 4)):
                                nc.gpsimd.dma_gather(
                                    g[:, s0 : s0 + ns, :, :].rearrange(
                                        "p s a c -> p s (a c)"
                                    ),
                                    xsrc,
                                    idxw[:, t, 128 * ch + 8 * s0 : 128 * ch + 8 * (s0 + ns)],
                                    num_idxs=ns * 128,
                                    num_idxs_reg=regs[ns],
                                    elem_size=256,
                                    elem_step=128,
                                    single_packet=False,
                                    queue_num=gi % 4,
                                )
                                gi += 1
                    return
                if "gonly2" in abl:
                    w4x, idxw = stage_a()
                    if "iotaidx" in abl:
                        idxw = apool.tile([128, KK, SLOTS * 8], I16, name="idxw2")
                        nc.gpsimd.iota(
                            idxw[:].rearrange("p a b -> p (a b)"),
                            pattern=[[3, KK * SLOTS * 8]],
                            base=0,
                            channel_multiplier=0,
                        )
                    xsrc = bass.AP(xp.tensor, 0, [[128, 2 * PIX], [1, 256]])
                    regs = {6: nc.gpsimd.to_reg(768), 4: nc.gpsimd.to_reg(512)}
                    gi = 0
                    for ch in range(NCHUNK):
                        for t in range(KK):
                            g = gpool.tile([128, CSLOT, 4, 64], F16, name="g")
                            for s0, ns in ((0, 6), (6, 6), (12, 4)):
                                nc.gpsimd.dma_gather(
                                    g[:, s0 : s0 + ns, :, :].rearrange(
                                        "p s a c -> p s (a c)"
                                    ),
                                    xsrc,
                                    idxw[:, t, 128 * ch + 8 * s0 : 128 * ch + 8 * (s0 + ns)],
                                    num_idxs=ns * 128,
                                    num_idxs_reg=regs[ns],
                                    elem_size=256,
                                    elem_step=128,
                                    single_packet=False,
                                    queue_num=gi % int(os.environ.get("NQ", "4")),
                                )
                                gi += 1
                    return
                if "noa" in abl:
                    # gather-only isolation: iota indices, no stage A
                    idxw = apool.tile([128, KK, SLOTS * 8], I16, name="idxw")
                    nc.gpsimd.iota(
                        idxw[:].rearrange("p a b -> p (a b)"),
                        pattern=[[3, KK * SLOTS * 8]],
                        base=0,
                        channel_multiplier=0,
                    )
                    w4 = None
                else:
                    w4, idxw = stage_a()
                main_loops(w4, idxw)

            def stage_a():
                if os.environ.get("STAGEA", "") == "1":
                    return stage_a_old()
                return stage_b()

            def stage_b():
                # ---- stage B: indices + bilinear weights, lean formulation.
                # Triangle weights: the interp weight of fetched column c is
                # max(0, 1 - |px - c|), which reproduces the reference's
                # zero-padding/clamp semantics with no eq-masking, and is
                # tolerant to an off-by-one window anchor (so the anchor cast
                # does not need exact floor semantics: cast(px-0.5) under
                # either trunc or round-to-nearest keeps the support inside
                # the fetched window).
                # Stored per corner: s = min(|d|,1) - 1 in [-1,0]; the product
                # of the two (col,row) s-factors equals the corner weight.
                OFF = apool.tile([128, SLOTS, 2 * KK], F32, name="OFF")
                nc.sync.dma_start(
                    OFF[:], offs.rearrange("(s p) c -> p s c", p=128)
                )
                vec = nc.vector
                shp = [128, SLOTS, KK]

                def atile(name):
                    return apool.tile(shp, F32, name=name)

                offx = atile("offx")
                vec.tensor_copy(offx[:], OFF[:, :, 0::2])
                offy = atile("offy")
                vec.tensor_copy(offy[:], OFF[:, :, 1::2])
                px = atile("px")
                vec.tensor_scalar(px[:], offx[:], xx[:, 0:1], None, Alu.add)
                py = atile("py")
                yyb = yy[:, :].unsqueeze(2).broadcast_to(shp)
                vec.tensor_tensor(py[:], offy[:], yyb, Alu.add)
                # window anchor: xc = clamp(int(px - 0.5), 0, 126)
                casti = apool.tile(shp, mybir.dt.int32, name="casti")
                vec.tensor_scalar(casti[:], px[:], -0.5, None, Alu.add)
                xc = atile("xc")
                vec.tensor_scalar(xc[:], casti[:], 0.0, 126.0, Alu.max, Alu.min)
                vec.tensor_scalar(casti[:], py[:], -0.5, None, Alu.add)
                yc = atile("yc")
                vec.tensor_scalar(yc[:], casti[:], 0.0, 126.0, Alu.max, Alu.min)
                # gather idx = yc*128 + xc  (row-pair interleaved layout)
                idx16 = apool.tile([128, KK, SLOTS], I16, name="idx16")
                vec.scalar_tensor_tensor(
                    idx16[:].rearrange("p t s -> p s t"),
                    yc[:], 128.0, xc[:], Alu.mult, Alu.add,
                )
                # s-factors per axis/side: s = min(|px - c|, 1) - 1
                dl = atile("dl")
                vec.tensor_tensor(dl[:], px[:], xc[:], Alu.subtract)
                dr = atile("dr")
                vec.tensor_scalar(dr[:], dl[:], 1.0, None, Alu.subtract)
                vec.scalar_tensor_tensor(dl[:], dl[:], -1.0, dl[:], Alu.mult, Alu.max)
                vec.tensor_scalar(dl[:], dl[:], 1.0, 1.0, Alu.min, Alu.subtract)
                vec.scalar_tensor_tensor(dr[:], dr[:], -1.0, dr[:], Alu.mult, Alu.max)
                vec.tensor_scalar(dr[:], dr[:], 1.0, 1.0, Alu.min, Alu.subtract)
                dt_ = atile("dt")
                vec.tensor_tensor(dt_[:], py[:], yc[:], Alu.subtract)
                db = atile("db")
                vec.tensor_scalar(db[:], dt_[:], 1.0, None, Alu.subtract)
                vec.scalar_tensor_tensor(dt_[:], dt_[:], -1.0, dt_[:], Alu.mult, Alu.max)
                vec.tensor_scalar(dt_[:], dt_[:], 1.0, 1.0, Alu.min, Alu.subtract)
                vec.scalar_tensor_tensor(db[:], db[:], -1.0, db[:], Alu.mult, Alu.max)
                vec.tensor_scalar(db[:], db[:], 1.0, 1.0, Alu.min, Alu.subtract)
                # corner weights [128, SLOTS*4, KK] f16, (slot,corner) merged
                # into one stride-9 dim so the per-(chunk,tap) multiply AP is
                # 3D (ScalarTensorTensor requires <=3D).
                # corner c: 0=(L,T) 1=(L,B) 2=(R,T) 3=(R,B)
                w4 = apool.tile([128, SLOTS * 4, KK], F16, name="w4")
                vec.tensor_tensor(w4[:, 0::4, :], dl[:], dt_[:], Alu.mult)
                vec.tensor_tensor(w4[:, 1::4, :], dl[:], db[:], Alu.mult)
                vec.tensor_tensor(w4[:, 2::4, :], dr[:], dt_[:], Alu.mult)
                vec.tensor_tensor(w4[:, 3::4, :], dr[:], db[:], Alu.mult)
                mmode = os.environ.get("MULT", "stt")
                wmul = w4

                # SWDGE wrapped-16 idx layout + replication to 128 partitions
                idxw = apool.tile([128, KK, SLOTS * 8], I16, name="idxw")
                idxwv = idxw[:].rearrange("p t (s k) -> p t s k", k=8)
                for k in range(8):
                    nc.sync.dma_start(
                        idxwv[0:16, :, :, k].squeeze(),
                        idx16[16 * k : 16 * (k + 1), :, :],
                    )
                span = 16
                while span < 128:
                    nc.sync.dma_start(
                        idxw[span : 2 * span, :, :],
                        idxw[0:span, :, :],
                    )
                    span *= 2
                return (mmode, wmul), idxw

            def stage_a_old():
                # ---- stage A: indices + weights, pipelined by tap-group ---
                # layout [128 part = pixel%128 (img col), slot = pixel//128
                # (img row), tap]. Computed in groups of 3 taps so the first
                # gathers can issue while the rest of stage A still runs.
                OFF = apool.tile([128, SLOTS, 2 * KK], F32, name="OFF")
                nc.sync.dma_start(
                    OFF[:], offs.rearrange("(s p) c -> p s c", p=128)
                )

                w4 = apool.tile([128, SLOTS, KK, 4], F16, name="w4")
                idx16 = apool.tile([128, KK, SLOTS], I16, name="idx16")
                idxw = apool.tile([128, KK, SLOTS * 8], I16, name="idxw")
                idxwv = idxw[:].rearrange("p t (s k) -> p t s k", k=8)
                tg = int(os.environ.get("TGRP", "3"))
                for t0 in range(0, KK, tg):
                    stage_a_group(OFF, w4, idx16, idxw, idxwv, t0,
                                  min(tg, KK - t0))
                return ("tt_old", w4), idxw

            def stage_a_group(OFF, w4, idx16, idxw, idxwv, t0, nt):
                offx = OFF[:, :, 2 * t0 : 2 * (t0 + nt) : 2]   # [128, 64, nt]
                offy = OFF[:, :, 2 * t0 + 1 : 2 * (t0 + nt) : 2]

                shp = [128, SLOTS, nt]

                def atile(name):
                    return apool.tile(shp, F32, name=name)

                vec = nc.vector
                px = atile("px")
                vec.tensor_scalar(px[:], offx, xx[:, 0:1], None, Alu.add)
                py = atile("py")
                yyb = yy[:, :].unsqueeze(2).broadcast_to(shp)
                vec.tensor_tensor(py[:], offy, yyb, Alu.add)
                # floor via int cast: F = round-ish(v) - (round-ish(v) > v)
                # exact for truncate or round-to-nearest cast semantics.
                casti = apool.tile(shp, mybir.dt.int32, name="casti")
                rnd = atile("rnd")
                tn = atile("tn")

                def floor_into(dst, v):
                    # dst <- floor(v)
                    vec.tensor_copy(casti[:], v[:])
                    vec.tensor_copy(rnd[:], casti[:])
                    vec.tensor_tensor(tn[:], rnd[:], v[:], Alu.is_gt)
                    vec.scalar_tensor_tensor(
                        dst[:], tn[:], -1.0, rnd[:], Alu.mult, Alu.add
                    )

                x0 = atile("x0")
                floor_into(x0, px)
                fx = atile("fx")
                vec.tensor_tensor(fx[:], px[:], x0[:], Alu.subtract)
                y0 = atile("y0")
                floor_into(y0, py)
                fy = atile("fy")
                vec.tensor_tensor(fy[:], py[:], y0[:], Alu.subtract)
                xc = atile("xc")
                vec.tensor_scalar(xc[:], x0[:], 0.0, 126.0, Alu.max, Alu.min)
                dx = atile("dx")
                vec.tensor_tensor(dx[:], xc[:], x0[:], Alu.subtract)
                g0 = atile("g0")
                vec.tensor_scalar(g0[:], y0[:], 0.0, 126.0, Alu.max, Alu.min)
                dy = atile("dy")
                vec.tensor_tensor(dy[:], g0[:], y0[:], Alu.subtract)
                # idx = g0*128 + xc (row-pair-per-row interleaved layout)
                idxf = atile("idxf")
                vec.tensor_scalar(idxf[:], g0[:], 128.0, None, Alu.mult)
                vec.tensor_tensor(idxf[:], idxf[:], xc[:], Alu.add)
                vec.tensor_copy(
                    idx16[:, t0 : t0 + nt, :].rearrange("p t s -> p s t"),
                    idxf[:],
                )

                # weights. wx1=fx, wx0=1-fx
                wx0 = atile("wx0")
                vec.tensor_scalar(wx0[:], fx[:], -1.0, 1.0, Alu.mult, Alu.add)
                wy0 = atile("wy0")
                vec.tensor_scalar(wy0[:], fy[:], -1.0, 1.0, Alu.mult, Alu.add)
                e0 = atile("e0")
                vec.tensor_scalar(e0[:], dx[:], 0.0, None, Alu.is_equal)
                e1 = atile("e1")
                vec.tensor_scalar(e1[:], dx[:], 1.0, None, Alu.is_equal)
                em = atile("em")
                vec.tensor_scalar(em[:], dx[:], -1.0, None, Alu.is_equal)
                # wcL = wx0*e0 + fx*e1 ; wcR = wx0*em + fx*e0
                wcl = atile("wcl")
                vec.tensor_tensor(wcl[:], wx0[:], e0[:], Alu.mult)
                vec.tensor_tensor(e1[:], fx[:], e1[:], Alu.mult)
                vec.tensor_tensor(wcl[:], wcl[:], e1[:], Alu.add)
                wcr = atile("wcr")
                vec.tensor_tensor(wcr[:], wx0[:], em[:], Alu.mult)
                vec.tensor_tensor(e0[:], fx[:], e0[:], Alu.mult)
                vec.tensor_tensor(wcr[:], wcr[:], e0[:], Alu.add)
                # rows
                r0 = atile("r0")
                vec.tensor_scalar(r0[:], dy[:], 0.0, None, Alu.is_equal)
                r1 = atile("r1")
                vec.tensor_scalar(r1[:], dy[:], 1.0, None, Alu.is_equal)
                rm = atile("rm")
                vec.tensor_scalar(rm[:], dy[:], -1.0, None, Alu.is_equal)
                wrt = atile("wrt")
                vec.tensor_tensor(wrt[:], wy0[:], r0[:], Alu.mult)
                vec.tensor_tensor(r1[:], fy[:], r1[:], Alu.mult)
                vec.tensor_tensor(wrt[:], wrt[:], r1[:], Alu.add)
                wrb = atile("wrb")
                vec.tensor_tensor(wrb[:], wy0[:], rm[:], Alu.mult)
                vec.tensor_tensor(r0[:], fy[:], r0[:], Alu.mult)
                vec.tensor_tensor(wrb[:], wrb[:], r0[:], Alu.add)
                # W4 [128, slot, tap, n] with n = (col, row):
                # 0=(L,T) 1=(L,B) 2=(R,T) 3=(R,B)
                wsl4 = w4[:, :, t0 : t0 + nt, :]
                vec.tensor_tensor(wsl4[:, :, :, 0], wcl[:], wrt[:], Alu.mult)
                vec.tensor_tensor(wsl4[:, :, :, 1], wcl[:], wrb[:], Alu.mult)
                vec.tensor_tensor(wsl4[:, :, :, 2], wcr[:], wrt[:], Alu.mult)
                vec.tensor_tensor(wsl4[:, :, :, 3], wcr[:], wrb[:], Alu.mult)

                # ---- idx shuffle into SWDGE wrapped-16 layout --------------
                # IDXW[q, t, s*8 + k] = idx16[16k+q, s, t]; replicated to all
                # eight 16-partition blocks.
                for k in range(8):
                    nc.sync.dma_start(
                        idxwv[0:16, t0 : t0 + nt, :, k].squeeze(),
                        idx16[16 * k : 16 * (k + 1), t0 : t0 + nt, :],
                    )
                # log-doubling replication: 16 -> 32 -> 64 -> 128 partitions
                span = 16
                while span < 128:
                    nc.sync.dma_start(
                        idxw[span : 2 * span, t0 : t0 + nt, :],
                        idxw[0:span, t0 : t0 + nt, :],
                    )
                    span *= 2

            def main_loops(w4, idxw):
                vec = nc.vector
                # gather source view: row r -> 256 contiguous floats starting
                # at r*128 (overlapping windows)
                xsrc = bass.AP(xp.tensor, 0, [[128, 2 * PIX], [1, 256]])  # fp16 rows
                gidx = [0]
                regs = {6: nc.gpsimd.to_reg(768), 4: nc.gpsimd.to_reg(512),
                        2: nc.gpsimd.to_reg(256), 16: nc.gpsimd.to_reg(2048),
                        8: nc.gpsimd.to_reg(1024)}

                # ---- main loop --------------------------------------------
                for ch in range(NCHUNK):
                    for t in range(KK):
                        sub = t % 2
                        pair = t // 2
                        g = gpool.tile([128, CSLOT, 4, 64], F16, name="g")
                        if "nogather" in abl and ch + t == 0:
                            nc.vector.memset(g[:], 0.25)
                        # sub-gathers sized to the SWDGE ring (<=768 descs)
                        if os.environ.get("SUBS", "664") == "664":
                            subs = ((0, 6), (6, 6), (12, 4))
                        elif os.environ.get("SUBS") == "16":
                            subs = ((0, 16),)
                        elif os.environ.get("SUBS") == "88":
                            subs = ((0, 8), (8, 8))
                        elif os.environ.get("SUBS") == "44":
                            subs = ((0, 4), (4, 4), (8, 4), (12, 4))
                        else:
                            subs = ((0, 2), (2, 2), (4, 2), (6, 2),
                                    (8, 2), (10, 2), (12, 2), (14, 2))
                        if "nogather" in abl:
                            subs = ()
                        for s0, ns in subs:
                            nidx = ns * 128
                            qn = gidx[0] % int(os.environ.get("NQ", "4"))
                            nc.gpsimd.dma_gather(
                                g[:, s0 : s0 + ns, :, :].rearrange(
                                    "p s a c -> p s (a c)"
                                ),
                                xsrc,
                                idxw[
                                    :, t,
                                    128 * ch + 8 * s0 : 128 * ch + 8 * (s0 + ns),
                                ],
                                num_idxs=nidx,
                                num_idxs_reg=regs[ns],
                                elem_size=256,
                                elem_step=128,
                                single_packet=False,
                                queue_num=qn,
                            )
                            gidx[0] += 1
                        # combine: g *= w4 (bcast over c); all 4 corner
                        # adds are folded into the conv matmul K-dim.
                        if "nodve" not in abl:
                            mmode, wt = w4
                            if mmode in ("stt", "tt"):
                                # all-SBUF 3D APs -> 2x_2p DVE mode for stt
                                gv = g[:].rearrange("p s c k -> p (s c) k")
                                wsl = (
                                    wt[:, ch * CSLOT * 4 : (ch + 1) * CSLOT * 4, t]
                                    .unsqueeze(2)
                                    .broadcast_to([128, CSLOT * 4, 64])
                                )
                                if mmode == "stt":
                                    vec.scalar_tensor_tensor(
                                        gv, gv, 1.0, wsl, Alu.mult, Alu.mult
                                    )
                                else:
                                    vec.tensor_tensor(gv, gv, wsl, Alu.mult)
                            else:
                                wsl = (
                                    wt[:, ch * CSLOT : (ch + 1) * CSLOT, t, :]
                                    .unsqueeze(3)
                                    .broadcast_to([128, CSLOT, 4, 64])
                                )
                                vec.tensor_tensor(g[:], g[:], wsl, Alu.mult)
                        if "nope" not in abl:
                            stm = stpool.tile([128, 2, CPIX], F16, name="stm")
                            for hb in range(2):
                                pt = ptpool.tile([128, CPIX], F16, name="pt")
                                for slot in range(CSLOT):
                                    nc.tensor.matmul(
                                        pt[:, 128 * slot : 128 * (slot + 1)],
                                        g[:, slot, 2 * hb : 2 * hb + 2, :],
                                        ident[:],
                                        is_transpose=True,
                                    )
                                nc.scalar.activation(
                                    stm[:, hb, :], pt[:], Act.Copy
                                )
                            if t == 0:
                                pc = pcpool.tile([OC, CPIX], F32, name="pc")
                            for hb in range(2):
                                for nb in range(CPIX // 512):
                                    nc.tensor.matmul(
                                        pc[:, 512 * nb : 512 * (nb + 1)],
                                        wpt[:, OC * t : OC * (t + 1)],
                                        stm[:, hb, 512 * nb : 512 * (nb + 1)],
                                        start=(t == 0 and hb == 0),
                                        stop=(t == KK - 1 and hb == 1),
                                    )
                    if "nope" in abl:
                        nc.gpsimd.dma_start(
                            out[0:64, CPIX * ch : CPIX * ch + 1024],
                            g[0:64, :, :, :].rearrange("p a b c -> p (a b c)")[
                                :, 0:1024
                            ],
                        )
                    else:
                        osb = opool.tile([OC, CPIX], F32, name="osb")
                        nc.scalar.activation(
                            osb[:], pc[:], Act.Identity, bias=bias[:, 0:1]
                        )
                        nc.sync.dma_start(
                            out[:, CPIX * ch : CPIX * (ch + 1)], osb[:]
                        )

            if loop_n > 0:
                with tc.For_i(0, loop_n, 1):
                    body()
            else:
                body()
            if "gonly" in abl or "gonly2" in abl:
                dummy = cpool.tile([OC, 64], F32)
                nc.vector.memset(dummy[:], 1.0)
                nc.sync.dma_start(out[:, 0:64], dummy[:])

    nc.compile()
    return nc


def prep_core_inputs(x, offset, weight, bias, core):
    """Host-side shard/layout prep for one core. Pure layout, no math on
    tensor values (beyond the reference-mandated reshape semantics)."""
    s, half = core // 2, core % 2
    # interleaved row-pair NHWC: xpair[par*PIX + pr*128 + j] =
    #   [x[2pr+par, j, :], x[2pr+par+1, j, :]]
    xr = np.ascontiguousarray(x[s].transpose(1, 2, 0))          # [H, W, C]
    xpad = np.concatenate([xr, np.zeros((1, W, C), np.float32)], 0)  # [129,W,C]
    rows = np.stack([xpad[0:128], xpad[1:129]], 1)           # [128, 2, W, C]
    # xpair row (g0, j) = [x[g0, j, :], x[g0+1, j, :]]
    xpair = np.concatenate(
        [rows.transpose(0, 2, 1, 3).reshape(2 * PIX, 128),
         np.zeros((1, 128), np.float32)], 0)

    # offsets for this half, pixel-major [PIX, 18]
    off = np.ascontiguousarray(
        offset[s, :, 64 * half : 64 * half + HALF, :]
        .transpose(1, 2, 0)
        .reshape(PIX, 2 * KK)
    )
    # channel c=2t is x-offset, 2t+1 is y-offset (reference reshape
    # [kk,2,h,w]: x = off[:,:,0], y = off[:,:,1] -> channel t*2+0 / t*2+1)

    # tap-pair weight slabs [128, 5*64]: rows sub*64+c, cols pair*64+o
    wfull = weight.reshape(OC, C, KK)
    wpair = np.zeros((128, KK * OC), np.float32)
    for t in range(KK):
        wt = wfull[:, :, t].T          # [c, o]
        wpair[0:64, t * OC : (t + 1) * OC] = wt
        wpair[64:128, t * OC : (t + 1) * OC] = wt

    yy = np.broadcast_to(
        (np.arange(SLOTS, dtype=np.float32) + 64 * half)[None, :], (128, SLOTS)
    ).copy()
    xxc = np.arange(128, dtype=np.float32).reshape(128, 1).copy()
    return {
        "xpair": np.ascontiguousarray(xpair).astype(np.float16),
        "offs": off.astype(np.float32),
        "wpair": wpair.astype(np.float16),
        "yy": yy,
        "xx": xxc,
        "ident": np.eye(128, dtype=np.float16),
        "bias": bias.reshape(OC, 1).astype(np.float32),
    }


_CACHE = {}


def kernel(x, offset, weight, bias):
    x = np.asarray(x, np.float32)
    offset = np.asarray(offset, np.float32)
    weight = np.asarray(weight, np.float32)
    bias = np.asarray(bias, np.float32)
    if "nc" not in _CACHE:
        _CACHE["nc"] = build_program()
    nc = _CACHE["nc"]
    in_maps = [prep_core_inputs(x, offset, weight, bias, c) for c in range(8)]
    res = run_bass_kernel_spmd(nc, in_maps, core_ids=list(range(8)))
    outf = np.empty((B, OC, H, W), np.float32)
    for c in range(8):
        s, half = c // 2, c % 2
        outf[s, :, 64 * half : 64 * half + HALF, :] = res.results[c][
            "out"
        ].reshape(OC, HALF, W)
    return outf



# revision 38
# speedup vs baseline: 1.9956x; 1.9956x over previous
"""Deformable Conv2d Lite (K=3) on 8 Trainium2 NeuronCores.

Sharding: data-parallel over batch x image-half. Core n handles sample n//2,
image rows [64*(n%2), 64*(n%2)+64). Weight replicated.

Device pipeline per core (~180-230us HW, rel err ~5.6e-4; the SWDGE gather
of 37.7MB/core is the pacing stage at ~185us, everything else overlaps it):
  1. Stage B (DVE, ~20 ops): from raw offsets compute per (tap, pixel) a
     gather index into a row-pair-interleaved fp16 NHWC layout of x (row r
     of xpair holds image rows r, r+1 at one column; idx = yc*128 + xc with
     anchor xc = clamp(int(px-0.5), 0, 126)), plus per-axis "s-factors"
     s = min(|px - c|, 1) - 1 packed per (L,R)/(T,B) pair; products of two
     s-factors equal the bilinear corner weights and reproduce the
     reference's zero-padding semantics with no eq-masking (triangle
     weights tolerate an off-by-one anchor, so the int cast needs no exact
     floor). Corner weights are expanded over channel PAIRS (wexp2
     [..,4,2]) so the main multiply has packed-inner APs on every operand
     (2x_1p DVE mode; broadcast stride-0 operands would force 1x).
  2. idx shuffle into the SWDGE wrapped-16 layout: 8 contiguous
     partition-block DMAs into a k-major staging tile + ONE DVE
     strided-read copy for the s*8+k interleave + 3 log-doubling
     replication DMAs. (Direct strided scatters emit 2-byte-run DMA
     descriptors - a ~137us descriptor bomb.)
  3. SWDGE dma_gather (fp16): one 512B descriptor per (tap, pixel) fetches
     the full 2x2 x 64ch bilinear patch from DRAM, round-robined over 4
     SWDGE queues.
  4. DVE per (chunk, tap): one packed in-place weight multiply (2x) and one
     packed add pre-summing L+R corners (c2, row pair kept separate); DVE
     also does the PSUM->SBUF copies (tensor_copy, 2x_1p from PSUM).
  5. PE: one fp16 [128,128] transpose per (slot, tap) to channel-major
     (contraction dim = 2 rows x 64ch = 128), then conv matmuls with
     per-tap weight slabs (W_t.T stacked twice, row weights already in the
     corner weights) accumulating f32 in PSUM over 9 taps.
  6. ACT: final bias-add on the conv PSUM; DMA out.

Steady-state software pipelining: loop bodies are emitted unrolled x2 with
double-buffered stage-B pools; a prologue computes iteration 1's
(weights, idxw) and each body recomputes the NEXT iteration's set (values
are identical across loop trips) so gathers always start from ready
buffers.
"""

import sys

for _p in ("/opt/trn_rl_repo",):
    if _p not in sys.path:
        sys.path.insert(0, _p)

import numpy as np

import concourse.bass as bass
import concourse.tile as tile
from concourse import bacc, mybir
from concourse.bass_utils import run_bass_kernel_spmd

F32 = mybir.dt.float32
F16 = mybir.dt.float16
I16 = mybir.dt.int16
Alu = mybir.AluOpType
Act = mybir.ActivationFunctionType

B, C, H, W = 4, 64, 128, 128
OC, KK = 64, 9
HALF = H // 2            # rows per core
PIX = HALF * W           # 8192 pixels per core
NCHUNK = 4
CPIX = PIX // NCHUNK     # 2048 pixels per chunk
CSLOT = CPIX // 128      # 16 slots per chunk
SLOTS = PIX // 128       # 64
NPAIR = 5                # ceil(9/2) tap pairs

XROWS = 2 * PIX + 1      # interleaved pair-row count incl. pad


import os


def build_program(loop_n: int = 0, ablate: str = ""):
    """Build the per-core Bass program. loop_n>0 wraps the body in a device
    For_i loop (for wall-clock timing); loop_n==0 emits the plain body.
    ablate: comma-set of {nogather, nodve, nope} for perf bisection."""
    abl = set(ablate.split(",")) if ablate else set()
    import os
    nc = bacc.Bacc("TRN2", target_bir_lowering=False, debug=False, num_devices=8,
                   num_swdge_queues=4,
                   dynamic_dma_scratch_size=int(os.environ.get("DDS", "16384")))

    xp = nc.dram_tensor("xpair", [XROWS, 128], F16, kind="ExternalInput").ap()
    offs = nc.dram_tensor("offs", [128, SLOTS * 2 * KK], F32,
                          kind="ExternalInput").ap()
    wp = nc.dram_tensor("wpair", [128, KK * OC], F16, kind="ExternalInput").ap()
    wp2 = nc.dram_tensor("wpair2", [128, NPAIR * OC], F16,
                         kind="ExternalInput").ap()
    yyd = nc.dram_tensor("yy", [128, SLOTS], F32, kind="ExternalInput").ap()
    xxd = nc.dram_tensor("xx", [128, 1], F32, kind="ExternalInput").ap()
    idd = nc.dram_tensor("ident", [128, 128], F16, kind="ExternalInput").ap()
    bsd = nc.dram_tensor("bias", [OC, 1], F32, kind="ExternalInput").ap()
    out = nc.dram_tensor("out", [OC, PIX], F32, kind="ExternalOutput").ap()

    with tile.TileContext(nc) as tc:
        import contextlib

        with contextlib.ExitStack() as ctx:
            cpool = ctx.enter_context(tc.tile_pool(name="consts", bufs=1))
            apool = ctx.enter_context(tc.tile_pool(name="stageA", bufs=1))
            bpool = ctx.enter_context(
                tc.tile_pool(name="stageB", bufs=int(os.environ.get("BBUFS", "2")))
            )
            gpool = ctx.enter_context(tc.tile_pool(name="gather", bufs=int(os.environ.get("GBUFS", "4"))))
            c2pool = ctx.enter_context(tc.tile_pool(name="csum", bufs=2))
            cppool = ctx.enter_context(tc.tile_pool(name="cpair", bufs=3))
            stpool = ctx.enter_context(tc.tile_pool(name="stmaj", bufs=int(os.environ.get("SBUFS", "4"))))
            opool = ctx.enter_context(tc.tile_pool(name="outsb", bufs=2))
            ptpool = ctx.enter_context(
                tc.tile_pool(name="psumT", bufs=2, space="PSUM")
            )
            pcpool = ctx.enter_context(
                tc.tile_pool(name="psumC", bufs=1, space="PSUM")
            )

            # ---- constants -------------------------------------------------
            xx = cpool.tile([128, 1], F32)
            nc.sync.dma_start(xx[:], xxd[:, :])
            yy = cpool.tile([128, SLOTS], F32)
            nc.sync.dma_start(yy[:], yyd[:, :])
            ident = cpool.tile([128, 128], F16)
            nc.sync.dma_start(ident[:], idd[:, :])
            wpt = cpool.tile([128, KK * OC], F16)
            nc.sync.dma_start(wpt[:], wp[:, :])
            wpt2 = cpool.tile([128, NPAIR * OC], F16)
            nc.sync.dma_start(wpt2[:], wp2[:, :])
            bias = cpool.tile([OC, 1], F32)
            nc.sync.dma_start(bias[:], bsd[:, :])
            wconst = None
            if int(os.environ.get("SBCUT", "0")) >= 2:
                if os.environ.get("COMB", "1") == "1p":
                    wconst = cpool.tile([128, NCHUNK, KK, CSLOT, 4, 2], F16)
                else:
                    wconst = cpool.tile([128, SLOTS * 4, KK, 2], F16)
                nc.vector.memset(wconst[:], 0.25)

            def body(_iv=None):
                if "gonly" in abl:
                    idxw = apool.tile([128, KK, SLOTS * 8], I16, name="idxw")
                    nc.gpsimd.iota(
                        idxw[:].rearrange("p a b -> p (a b)"),
                        pattern=[[3, KK * SLOTS * 8]],
                        base=0,
                        channel_multiplier=0,
                    )
                    xsrc = bass.AP(xp.tensor, 0, [[128, 2 * PIX], [1, 256]])
                    regs = {6: nc.gpsimd.to_reg(768), 4: nc.gpsimd.to_reg(512)}
                    gi = 0
                    for ch in range(NCHUNK):
                        for t in range(KK):
                            g = gpool.tile([128, CSLOT, 4, 64], F16, name="g")
                            for s0, ns in ((0, 6), (6, 6), (12, 4)):
                                nc.gpsimd.dma_gather(
                                    g[:, s0 : s0 + ns, :, :].rearrange(
                                        "p s a c -> p s (a c)"
                                    ),
                                    xsrc,
                                    idxw[:, t, 128 * ch + 8 * s0 : 128 * ch + 8 * (s0 + ns)],
                                    num_idxs=ns * 128,
                                    num_idxs_reg=regs[ns],
                                    elem_size=256,
                                    elem_step=128,
                                    single_packet=False,
                                    queue_num=gi % 4,
                                )
                                gi += 1
                    return
                if "gonly2" in abl:
                    w4x, idxw = stage_a()
                    if "iotaidx" in abl:
                        idxw = apool.tile([128, KK, SLOTS * 8], I16, name="idxw2")
                        nc.gpsimd.iota(
                            idxw[:].rearrange("p a b -> p (a b)"),
                            pattern=[[3, KK * SLOTS * 8]],
                            base=0,
                            channel_multiplier=0,
                        )
                    xsrc = bass.AP(xp.tensor, 0, [[128, 2 * PIX], [1, 256]])
                    regs = {6: nc.gpsimd.to_reg(768), 4: nc.gpsimd.to_reg(512)}
                    gi = 0
                    for ch in range(NCHUNK):
                        for t in range(KK):
                            g = gpool.tile([128, CSLOT, 4, 64], F16, name="g")
                            for s0, ns in ((0, 6), (6, 6), (12, 4)):
                                nc.gpsimd.dma_gather(
                                    g[:, s0 : s0 + ns, :, :].rearrange(
                                        "p s a c -> p s (a c)"
                                    ),
                                    xsrc,
                                    idxw[:, t, 128 * ch + 8 * s0 : 128 * ch + 8 * (s0 + ns)],
                                    num_idxs=ns * 128,
                                    num_idxs_reg=regs[ns],
                                    elem_size=256,
                                    elem_step=128,
                                    single_packet=False,
                                    queue_num=gi % int(os.environ.get("NQ", "4")),
                                )
                                gi += 1
                    return
                w4, idxw = stage_a()
                main_loops(w4, idxw)

            def stage_a():
                if "nob" in abl:
                    # no-stage-B ablation: iota indices + constant weights
                    idxw = bpool.tile([128, KK, SLOTS * 8], I16, name="idxw")
                    nc.gpsimd.iota(
                        idxw[:].rearrange("p a b -> p (a b)"),
                        pattern=[[3, KK * SLOTS * 8]],
                        base=0,
                        channel_multiplier=0,
                    )
                    if os.environ.get("COMB", "1") == "1p":
                        wm = bpool.tile([128, NCHUNK, KK, CSLOT, 4, 2], F16,
                                        name="wexp5")
                    else:
                        wm = bpool.tile([128, SLOTS * 4, KK, 2], F16,
                                        name="wexp2")
                    nc.vector.memset(wm[:], 0.25)
                    return ("tt2", wm), idxw
                if os.environ.get("STAGEA", "") == "1":
                    return stage_a_old()
                return stage_b()

            def stage_b():
                # ---- stage B: indices + bilinear weights, lean formulation.
                # Triangle weights: the interp weight of fetched column c is
                # max(0, 1 - |px - c|), which reproduces the reference's
                # zero-padding/clamp semantics with no eq-masking, and is
                # tolerant to an off-by-one window anchor (so the anchor cast
                # does not need exact floor semantics: cast(px-0.5) under
                # either trunc or round-to-nearest keeps the support inside
                # the fetched window).
                # Stored per corner: s = min(|d|,1) - 1 in [-1,0]; the product
                # of the two (col,row) s-factors equals the corner weight.
                sbcut = int(os.environ.get("SBCUT", "0"))
                OFF = apool.tile([128, SLOTS, 2 * KK], F32, name="OFF")
                nc.sync.dma_start(
                    OFF[:], offs.rearrange("p (s c) -> p s c", c=2 * KK)
                )
                vec = nc.vector
                shp = [128, SLOTS, KK]

                def atile(name):
                    return apool.tile(shp, F32, name=name)

                def iota_idxw():
                    idxw = bpool.tile([128, KK, SLOTS * 8], I16, name="idxw")
                    nc.gpsimd.iota(
                        idxw[:].rearrange("p a b -> p (a b)"),
                        pattern=[[3, KK * SLOTS * 8]],
                        base=0,
                        channel_multiplier=0,
                    )
                    return idxw

                px = atile("px")
                vec.tensor_scalar(px[:], OFF[:, :, 0::2], xx[:, 0:1], None,
                                  Alu.add)
                py = atile("py")
                yyb = yy[:, :].unsqueeze(2).broadcast_to(shp)
                vec.tensor_tensor(py[:], OFF[:, :, 1::2], yyb, Alu.add)
                if sbcut >= 4:
                    return ("tt2", wconst), iota_idxw()
                # window anchors packed (c, c+1) per axis:
                # bx[..,0]=xc, bx[..,1]=xc+1 with xc = clamp(int(px-0.5),0,126)
                shp2 = [128, SLOTS, KK, 2]
                casti = apool.tile(shp, mybir.dt.int32, name="casti")
                bx = apool.tile(shp2, F32, name="bx")
                by = apool.tile(shp2, F32, name="by")
                vec.tensor_scalar(casti[:], px[:], -0.5, None, Alu.add)
                vec.tensor_scalar(bx[:, :, :, 0], casti[:], 0.0, 126.0,
                                  Alu.max, Alu.min)
                vec.tensor_scalar(bx[:, :, :, 1], bx[:, :, :, 0], 1.0, None,
                                  Alu.add)
                vec.tensor_scalar(casti[:], py[:], -0.5, None, Alu.add)
                vec.tensor_scalar(by[:, :, :, 0], casti[:], 0.0, 126.0,
                                  Alu.max, Alu.min)
                vec.tensor_scalar(by[:, :, :, 1], by[:, :, :, 0], 1.0, None,
                                  Alu.add)
                # gather idx = yc*128 + xc  (row-pair interleaved layout)
                idx16 = apool.tile([128, KK, SLOTS], I16, name="idx16")
                vec.scalar_tensor_tensor(
                    idx16[:].rearrange("p t s -> p s t"),
                    by[:, :, :, 0], 128.0, bx[:, :, :, 0], Alu.mult, Alu.add,
                )
                if sbcut >= 3:
                    return ("tt2", wconst), (
                        iota_idxw() if sbcut >= 1 else shuffle_idx(idx16)
                    )
                # s-factors, in place: s = min(|px - c|, 1) - 1  (in [-1,0];
                # products of two s-factors equal the bilinear corner weights)
                vec.tensor_tensor(
                    bx[:], px[:].unsqueeze(3).broadcast_to(shp2), bx[:],
                    Alu.subtract,
                )
                vec.scalar_tensor_tensor(bx[:], bx[:], -1.0, bx[:],
                                         Alu.mult, Alu.max)
                vec.tensor_scalar(bx[:], bx[:], 1.0, 1.0, Alu.min,
                                  Alu.subtract)
                vec.tensor_tensor(
                    by[:], py[:].unsqueeze(3).broadcast_to(shp2), by[:],
                    Alu.subtract,
                )
                vec.scalar_tensor_tensor(by[:], by[:], -1.0, by[:],
                                         Alu.mult, Alu.max)
                vec.tensor_scalar(by[:], by[:], 1.0, 1.0, Alu.min,
                                  Alu.subtract)
                # corner weights [128, SLOTS*4, KK] f16, (slot,corner) merged
                # into one stride-9 dim so broadcast-weight multiply APs stay
                # 3D. corner c = cx*2 + cy: 0=(L,T) 1=(L,B) 2=(R,T) 3=(R,B)
                w4 = bpool.tile([128, SLOTS * 4, KK], F16, name="w4")
                vec.tensor_tensor(
                    w4[:].rearrange("p (s cx cy) t -> p s t cx cy",
                                    cx=2, cy=2),
                    bx[:].unsqueeze(4).broadcast_to([128, SLOTS, KK, 2, 2]),
                    by[:].unsqueeze(3).broadcast_to([128, SLOTS, KK, 2, 2]),
                    Alu.mult,
                )
                mmode = os.environ.get("MULT", "tt2")
                if sbcut >= 2:
                    wmul = wconst
                    mmode = "tt2"
                elif os.environ.get("COMB", "1") == "1p":
                    # chunk-major tap-major channel-pair-expanded weights for
                    # pair-merged multiplies: [128, NCHUNK, KK, CSLOT, 4, 2];
                    # a (ch, tap-pair) slice is fully contiguous -> 3D AP.
                    wexp5 = bpool.tile([128, NCHUNK, KK, CSLOT, 4, 2], F16,
                                       name="wexp5")
                    for ck in range(NCHUNK):
                        vec.tensor_copy(
                            wexp5[:, ck],
                            w4[:, ck * CSLOT * 4 : (ck + 1) * CSLOT * 4, :]
                            .rearrange("p (s c) t -> p t s c", c=4)
                            .unsqueeze(4)
                            .broadcast_to([128, KK, CSLOT, 4, 2]),
                        )
                    wmul = wexp5
                elif mmode == "tt2":
                    # channel-pair-expanded weights: every AP of the main
                    # multiply is then packed-inner -> 2x_1p tensor_tensor.
                    wexp2 = bpool.tile([128, SLOTS * 4, KK, 2], F16,
                                       name="wexp2")
                    vec.tensor_copy(
                        wexp2[:],
                        w4[:].unsqueeze(3)
                        .broadcast_to([128, SLOTS * 4, KK, 2]),
                    )
                    wmul = wexp2
                else:
                    wmul = w4

                idxw = iota_idxw() if sbcut >= 1 else shuffle_idx(idx16)
                return (mmode, wmul), idxw

            def shuffle_idx(idx16):
                # SWDGE wrapped-16 idx layout + replication to 128 partitions.
                idxw = bpool.tile([128, KK, SLOTS * 8], I16, name="idxw")
                if os.environ.get("SHUF", "2") == "2":
                    # contiguous partition-block scatters into a k-major
                    # staging tile (cheap big-run descriptors), then ONE DVE
                    # strided-read copy makes the s*8+k interleave. The old
                    # direct-scatter wrote 2-byte-run descriptors (a ~137us
                    # descriptor bomb on HW).
                    tmp = apool.tile([16, 8, KK, SLOTS], I16, name="tmpw")
                    for k in range(8):
                        eng = nc.sync if k % 2 == 0 else nc.scalar
                        eng.dma_start(
                            tmp[0:16, k], idx16[16 * k : 16 * (k + 1), :, :]
                        )
                    vec = nc.vector
                    vec.tensor_copy(
                        idxw[0:16, :, :].rearrange(
                            "p t (s k) -> p t s k", k=8),
                        tmp[:].rearrange("p k t s -> p t s k"),
                    )
                else:
                    idxwv = idxw[:].rearrange("p t (s k) -> p t s k", k=8)
                    for k in range(8):
                        nc.sync.dma_start(
                            idxwv[0:16, :, :, k].squeeze(),
                            idx16[16 * k : 16 * (k + 1), :, :],
                        )
                span = 16
                while span < 128:
                    nc.sync.dma_start(
                        idxw[span : 2 * span, :, :],
                        idxw[0:span, :, :],
                    )
                    span *= 2
                return idxw

            def stage_a_old():
                # ---- stage A: indices + weights, pipelined by tap-group ---
                # layout [128 part = pixel%128 (img col), slot = pixel//128
                # (img row), tap]. Computed in groups of 3 taps so the first
                # gathers can issue while the rest of stage A still runs.
                OFF = apool.tile([128, SLOTS, 2 * KK], F32, name="OFF")
                nc.sync.dma_start(
                    OFF[:], offs.rearrange("p (s c) -> p s c", c=2 * KK)
                )

                w4 = apool.tile([128, SLOTS, KK, 4], F16, name="w4")
                idx16 = apool.tile([128, KK, SLOTS], I16, name="idx16")
                idxw = apool.tile([128, KK, SLOTS * 8], I16, name="idxw")
                idxwv = idxw[:].rearrange("p t (s k) -> p t s k", k=8)
                tg = int(os.environ.get("TGRP", "3"))
                for t0 in range(0, KK, tg):
                    stage_a_group(OFF, w4, idx16, idxw, idxwv, t0,
                                  min(tg, KK - t0))
                return ("tt_old", w4), idxw

            def stage_a_group(OFF, w4, idx16, idxw, idxwv, t0, nt):
                offx = OFF[:, :, 2 * t0 : 2 * (t0 + nt) : 2]   # [128, 64, nt]
                offy = OFF[:, :, 2 * t0 + 1 : 2 * (t0 + nt) : 2]

                shp = [128, SLOTS, nt]

                def atile(name):
                    return apool.tile(shp, F32, name=name)

                vec = nc.vector
                px = atile("px")
                vec.tensor_scalar(px[:], offx, xx[:, 0:1], None, Alu.add)
                py = atile("py")
                yyb = yy[:, :].unsqueeze(2).broadcast_to(shp)
                vec.tensor_tensor(py[:], offy, yyb, Alu.add)
                # floor via int cast: F = round-ish(v) - (round-ish(v) > v)
                # exact for truncate or round-to-nearest cast semantics.
                casti = apool.tile(shp, mybir.dt.int32, name="casti")
                rnd = atile("rnd")
                tn = atile("tn")

                def floor_into(dst, v):
                    # dst <- floor(v)
                    vec.tensor_copy(casti[:], v[:])
                    vec.tensor_copy(rnd[:], casti[:])
                    vec.tensor_tensor(tn[:], rnd[:], v[:], Alu.is_gt)
                    vec.scalar_tensor_tensor(
                        dst[:], tn[:], -1.0, rnd[:], Alu.mult, Alu.add
                    )

                x0 = atile("x0")
                floor_into(x0, px)
                fx = atile("fx")
                vec.tensor_tensor(fx[:], px[:], x0[:], Alu.subtract)
                y0 = atile("y0")
                floor_into(y0, py)
                fy = atile("fy")
                vec.tensor_tensor(fy[:], py[:], y0[:], Alu.subtract)
                xc = atile("xc")
                vec.tensor_scalar(xc[:], x0[:], 0.0, 126.0, Alu.max, Alu.min)
                dx = atile("dx")
                vec.tensor_tensor(dx[:], xc[:], x0[:], Alu.subtract)
                g0 = atile("g0")
                vec.tensor_scalar(g0[:], y0[:], 0.0, 126.0, Alu.max, Alu.min)
                dy = atile("dy")
                vec.tensor_tensor(dy[:], g0[:], y0[:], Alu.subtract)
                # idx = g0*128 + xc (row-pair-per-row interleaved layout)
                idxf = atile("idxf")
                vec.tensor_scalar(idxf[:], g0[:], 128.0, None, Alu.mult)
                vec.tensor_tensor(idxf[:], idxf[:], xc[:], Alu.add)
                vec.tensor_copy(
                    idx16[:, t0 : t0 + nt, :].rearrange("p t s -> p s t"),
                    idxf[:],
                )

                # weights. wx1=fx, wx0=1-fx
                wx0 = atile("wx0")
                vec.tensor_scalar(wx0[:], fx[:], -1.0, 1.0, Alu.mult, Alu.add)
                wy0 = atile("wy0")
                vec.tensor_scalar(wy0[:], fy[:], -1.0, 1.0, Alu.mult, Alu.add)
                e0 = atile("e0")
                vec.tensor_scalar(e0[:], dx[:], 0.0, None, Alu.is_equal)
                e1 = atile("e1")
                vec.tensor_scalar(e1[:], dx[:], 1.0, None, Alu.is_equal)
                em = atile("em")
                vec.tensor_scalar(em[:], dx[:], -1.0, None, Alu.is_equal)
                # wcL = wx0*e0 + fx*e1 ; wcR = wx0*em + fx*e0
                wcl = atile("wcl")
                vec.tensor_tensor(wcl[:], wx0[:], e0[:], Alu.mult)
                vec.tensor_tensor(e1[:], fx[:], e1[:], Alu.mult)
                vec.tensor_tensor(wcl[:], wcl[:], e1[:], Alu.add)
                wcr = atile("wcr")
                vec.tensor_tensor(wcr[:], wx0[:], em[:], Alu.mult)
                vec.tensor_tensor(e0[:], fx[:], e0[:], Alu.mult)
                vec.tensor_tensor(wcr[:], wcr[:], e0[:], Alu.add)
                # rows
                r0 = atile("r0")
                vec.tensor_scalar(r0[:], dy[:], 0.0, None, Alu.is_equal)
                r1 = atile("r1")
                vec.tensor_scalar(r1[:], dy[:], 1.0, None, Alu.is_equal)
                rm = atile("rm")
                vec.tensor_scalar(rm[:], dy[:], -1.0, None, Alu.is_equal)
                wrt = atile("wrt")
                vec.tensor_tensor(wrt[:], wy0[:], r0[:], Alu.mult)
                vec.tensor_tensor(r1[:], fy[:], r1[:], Alu.mult)
                vec.tensor_tensor(wrt[:], wrt[:], r1[:], Alu.add)
                wrb = atile("wrb")
                vec.tensor_tensor(wrb[:], wy0[:], rm[:], Alu.mult)
                vec.tensor_tensor(r0[:], fy[:], r0[:], Alu.mult)
                vec.tensor_tensor(wrb[:], wrb[:], r0[:], Alu.add)
                # W4 [128, slot, tap, n] with n = (col, row):
                # 0=(L,T) 1=(L,B) 2=(R,T) 3=(R,B)
                wsl4 = w4[:, :, t0 : t0 + nt, :]
                vec.tensor_tensor(wsl4[:, :, :, 0], wcl[:], wrt[:], Alu.mult)
                vec.tensor_tensor(wsl4[:, :, :, 1], wcl[:], wrb[:], Alu.mult)
                vec.tensor_tensor(wsl4[:, :, :, 2], wcr[:], wrt[:], Alu.mult)
                vec.tensor_tensor(wsl4[:, :, :, 3], wcr[:], wrb[:], Alu.mult)

                # ---- idx shuffle into SWDGE wrapped-16 layout --------------
                # IDXW[q, t, s*8 + k] = idx16[16k+q, s, t]; replicated to all
                # eight 16-partition blocks.
                for k in range(8):
                    nc.sync.dma_start(
                        idxwv[0:16, t0 : t0 + nt, :, k].squeeze(),
                        idx16[16 * k : 16 * (k + 1), t0 : t0 + nt, :],
                    )
                # log-doubling replication: 16 -> 32 -> 64 -> 128 partitions
                span = 16
                while span < 128:
                    nc.sync.dma_start(
                        idxw[span : 2 * span, t0 : t0 + nt, :],
                        idxw[0:span, t0 : t0 + nt, :],
                    )
                    span *= 2

            def main_loops(w4, idxw):
                vec = nc.vector
                # gather source view: row r -> 256 contiguous floats starting
                # at r*128 (overlapping windows)
                xsrc = bass.AP(xp.tensor, 0, [[128, 2 * PIX], [1, 256]])  # fp16 rows
                gidx = [0]
                cidx = [0]
                copydve = int(os.environ.get("COPYDVE", "1"))
                regs = {6: nc.gpsimd.to_reg(768), 4: nc.gpsimd.to_reg(512),
                        2: nc.gpsimd.to_reg(256), 16: nc.gpsimd.to_reg(2048),
                        8: nc.gpsimd.to_reg(1024)}
                comb = os.environ.get("COMB", "1")
                mmode, wt = w4
                if mmode == "tt_old":
                    comb = "0"

                def psum_to_sbuf(dst, src):
                    cidx[0] += 1
                    if copydve and cidx[0] % copydve == 0:
                        vec.tensor_copy(dst, src)
                    else:
                        nc.scalar.activation(dst, src, Act.Copy)

                def subs_for():
                    if os.environ.get("SUBS", "664") == "664":
                        return ((0, 6), (6, 6), (12, 4))
                    elif os.environ.get("SUBS") == "16":
                        return ((0, 16),)
                    elif os.environ.get("SUBS") == "88":
                        return ((0, 8), (8, 8))
                    elif os.environ.get("SUBS") == "44":
                        return ((0, 4), (4, 4), (8, 4), (12, 4))
                    return ((0, 2), (2, 2), (4, 2), (6, 2),
                            (8, 2), (10, 2), (12, 2), (14, 2))

                def gather_tap(dst3, t, ch):
                    # dst3: [128, CSLOT, 256]-shaped contiguous AP target
                    for s0, ns in () if "nogather" in abl else subs_for():
                        nidx = ns * 128
                        qn = gidx[0] % int(os.environ.get("NQ", "4"))
                        nc.gpsimd.dma_gather(
                            dst3[:, s0 : s0 + ns, :],
                            xsrc,
                            idxw[
                                :, t,
                                128 * ch + 8 * s0 : 128 * ch + 8 * (s0 + ns),
                            ],
                            num_idxs=nidx,
                            num_idxs_reg=regs[ns],
                            elem_size=256,
                            elem_step=128,
                            single_packet=os.environ.get("SPKT", "0") == "1",
                            queue_num=qn,
                        )
                        gidx[0] += 1

                def chunk_pairs(ch):
                    # pair-merged path: 2 taps per gather tile, one multiply
                    # and one corner-sum per pair; transposes/conv per tap.
                    if "nope" not in abl:
                        pc = pcpool.tile([OC, CPIX], F32, name="pc")
                    for pr in range(NPAIR):
                        ntp = 2 if 2 * pr + 1 < KK else 1
                        g2 = gpool.tile([128, 2, CSLOT, 4, 64], F16,
                                        name="g2")
                        if "nogather" in abl and ch + pr == 0:
                            nc.vector.memset(g2[:], 0.25)
                        for tp in range(ntp):
                            gather_tap(
                                g2[:, tp].rearrange("p s a c -> p s (a c)"),
                                2 * pr + tp, ch,
                            )
                        if "nodve" not in abl:
                            gv = g2[:, 0:ntp].rearrange(
                                "p t s c (h two) -> p (t s c) h two", two=2
                            )
                            wsl = (
                                wt[:, ch, 2 * pr : 2 * pr + ntp]
                                .rearrange("p t s c k -> p (t s c) k")
                                .unsqueeze(2)
                                .broadcast_to([128, ntp * CSLOT * 4, 32, 2])
                            )
                            vec.tensor_tensor(gv, gv, wsl, Alu.mult)
                        c2p = c2pool.tile([128, 2, CSLOT, 2, 64], F16,
                                          name="c2p")
                        if "nope" in abl and "nodve" in abl:
                            continue
                        vec.tensor_tensor(
                            c2p[:, 0:ntp].rearrange("p t s r k -> p (t s) (r k)"),
                            g2[:, 0:ntp, :, 0:2, :].rearrange(
                                "p t s r k -> p (t s) (r k)"),
                            g2[:, 0:ntp, :, 2:4, :].rearrange(
                                "p t s r k -> p (t s) (r k)"),
                            Alu.add,
                        )
                        if "nope" in abl:
                            continue
                        for tp in range(ntp):
                            t = 2 * pr + tp
                            stm = stpool.tile([128, CPIX], F16, name="stm")
                            pt = ptpool.tile([128, CPIX], F16, name="pt")
                            for slot in range(CSLOT):
                                nc.tensor.matmul(
                                    pt[:, 128 * slot : 128 * (slot + 1)],
                                    c2p[:, tp, slot, :, :],
                                    ident[:],
                                    is_transpose=True,
                                )
                            psum_to_sbuf(stm[:], pt[:])
                            for nb in range(CPIX // 512):
                                nc.tensor.matmul(
                                    pc[:, 512 * nb : 512 * (nb + 1)],
                                    wpt[:, OC * t : OC * (t + 1)],
                                    stm[:, 512 * nb : 512 * (nb + 1)],
                                    start=(t == 0),
                                    stop=(t == KK - 1),
                                )
                    if "nope" in abl:
                        nc.gpsimd.dma_start(
                            out[0:64, CPIX * ch : CPIX * ch + 1024],
                            g2[0:64, 0].rearrange("p a b c -> p (a b c)")[
                                :, 0:1024
                            ],
                        )
                        return
                    osb = opool.tile([OC, CPIX], F32, name="osb")
                    nc.scalar.activation(
                        osb[:], pc[:], Act.Identity, bias=bias[:, 0:1]
                    )
                    nc.sync.dma_start(
                        out[:, CPIX * ch : CPIX * (ch + 1)], osb[:]
                    )

                if comb == "1p":
                    for ch in range(NCHUNK):
                        chunk_pairs(ch)
                    return

                # ---- main loop --------------------------------------------
                for ch in range(NCHUNK):
                    cpair = None
                    for t in range(KK):
                        sub = t % 2
                        pair = t // 2
                        g = gpool.tile([128, CSLOT, 4, 64], F16, name="g")
                        if "nogather" in abl and ch + t == 0:
                            nc.vector.memset(g[:], 0.25)
                        # sub-gathers sized to the SWDGE ring (<=768 descs)
                        if os.environ.get("SUBS", "664") == "664":
                            subs = ((0, 6), (6, 6), (12, 4))
                        elif os.environ.get("SUBS") == "16":
                            subs = ((0, 16),)
                        elif os.environ.get("SUBS") == "88":
                            subs = ((0, 8), (8, 8))
                        elif os.environ.get("SUBS") == "44":
                            subs = ((0, 4), (4, 4), (8, 4), (12, 4))
                        else:
                            subs = ((0, 2), (2, 2), (4, 2), (6, 2),
                                    (8, 2), (10, 2), (12, 2), (14, 2))
                        if "nogather" in abl:
                            subs = ()
                        for s0, ns in subs:
                            nidx = ns * 128
                            qn = gidx[0] % int(os.environ.get("NQ", "4"))
                            nc.gpsimd.dma_gather(
                                g[:, s0 : s0 + ns, :, :].rearrange(
                                    "p s a c -> p s (a c)"
                                ),
                                xsrc,
                                idxw[
                                    :, t,
                                    128 * ch + 8 * s0 : 128 * ch + 8 * (s0 + ns),
                                ],
                                num_idxs=nidx,
                                num_idxs_reg=regs[ns],
                                elem_size=256,
                                elem_step=128,
                                single_packet=False,
                                queue_num=qn,
                            )
                            gidx[0] += 1
                        # per-corner bilinear weight multiply (bcast over ch
                        # unless tt2's packed channel-pair expansion is used)
                        if "nodve" not in abl:
                            if mmode == "tt2":
                                gv = g[:].rearrange(
                                    "p s c (h two) -> p (s c) h two", two=2
                                )
                                wsl = (
                                    wt[:, ch * CSLOT * 4 : (ch + 1) * CSLOT * 4,
                                       t, :]
                                    .unsqueeze(2)
                                    .broadcast_to([128, CSLOT * 4, 32, 2])
                                )
                                vec.tensor_tensor(gv, gv, wsl, Alu.mult)
                            elif mmode in ("stt", "tt"):
                                gv = g[:].rearrange("p s c k -> p (s c) k")
                                wsl = (
                                    wt[:, ch * CSLOT * 4 : (ch + 1) * CSLOT * 4, t]
                                    .unsqueeze(2)
                                    .broadcast_to([128, CSLOT * 4, 64])
                                )
                                if mmode == "stt":
                                    vec.scalar_tensor_tensor(
                                        gv, gv, 1.0, wsl, Alu.mult, Alu.mult
                                    )
                                else:
                                    vec.tensor_tensor(gv, gv, wsl, Alu.mult)
                            else:
                                wsl = (
                                    wt[:, ch * CSLOT : (ch + 1) * CSLOT, t, :]
                                    .unsqueeze(3)
                                    .broadcast_to([128, CSLOT, 4, 64])
                                )
                                vec.tensor_tensor(g[:], g[:], wsl, Alu.mult)
                        if "nope" in abl:
                            continue
                        if t == 0:
                            pc = pcpool.tile([OC, CPIX], F32, name="pc")
                        if comb == "0":
                            stm = stpool.tile([128, 2, CPIX], F16, name="stm")
                            for hb in range(2):
                                pt = ptpool.tile([128, CPIX], F16, name="pt")
                                for slot in range(CSLOT):
                                    nc.tensor.matmul(
                                        pt[:, 128 * slot : 128 * (slot + 1)],
                                        g[:, slot, 2 * hb : 2 * hb + 2, :],
                                        ident[:],
                                        is_transpose=True,
                                    )
                                psum_to_sbuf(stm[:, hb, :], pt[:])
                            for hb in range(2):
                                for nb in range(CPIX // 512):
                                    nc.tensor.matmul(
                                        pc[:, 512 * nb : 512 * (nb + 1)],
                                        wpt[:, OC * t : OC * (t + 1)],
                                        stm[:, hb, 512 * nb : 512 * (nb + 1)],
                                        start=(t == 0 and hb == 0),
                                        stop=(t == KK - 1 and hb == 1),
                                    )
                        elif comb == "1":
                            # pre-sum L+R corners (packed adds, 2x): rows stay
                            # separate -> contraction (row, ch) = 128 per tap
                            c2 = c2pool.tile([128, CSLOT, 2, 64], F16,
                                             name="c2")
                            vec.tensor_tensor(
                                c2[:].rearrange("p s r k -> p s (r k)"),
                                g[:, :, 0:2, :].rearrange(
                                    "p s r k -> p s (r k)"),
                                g[:, :, 2:4, :].rearrange(
                                    "p s r k -> p s (r k)"),
                                Alu.add,
                            )
                            stm = stpool.tile([128, CPIX], F16, name="stm")
                            pt = ptpool.tile([128, CPIX], F16, name="pt")
                            for slot in range(CSLOT):
                                nc.tensor.matmul(
                                    pt[:, 128 * slot : 128 * (slot + 1)],
                                    c2[:, slot, :, :],
                                    ident[:],
                                    is_transpose=True,
                                )
                            psum_to_sbuf(stm[:], pt[:])
                            for nb in range(CPIX // 512):
                                nc.tensor.matmul(
                                    pc[:, 512 * nb : 512 * (nb + 1)],
                                    wpt[:, OC * t : OC * (t + 1)],
                                    stm[:, 512 * nb : 512 * (nb + 1)],
                                    start=(t == 0),
                                    stop=(t == KK - 1),
                                )
                        else:
                            # full bilinear combine + 2-tap packing:
                            # contraction (tap-pair, ch) = 128 per pass
                            c2 = c2pool.tile([128, CSLOT, 2, 64], F16,
                                             name="c2")
                            vec.tensor_tensor(
                                c2[:].rearrange("p s r k -> p s (r k)"),
                                g[:, :, 0:2, :].rearrange(
                                    "p s r k -> p s (r k)"),
                                g[:, :, 2:4, :].rearrange(
                                    "p s r k -> p s (r k)"),
                                Alu.add,
                            )
                            if sub == 0:
                                cpair = cppool.tile([128, CSLOT, 2, 64], F16,
                                                    name="cpair")
                            vec.tensor_tensor(
                                cpair[:, :, sub, :],
                                c2[:, :, 0, :],
                                c2[:, :, 1, :],
                                Alu.add,
                            )
                            if sub == 1 or t == KK - 1:
                                single = sub == 0
                                stm = stpool.tile([128, CPIX], F16, name="stm")
                                pt = ptpool.tile([128, CPIX], F16, name="pt")
                                for slot in range(CSLOT):
                                    nc.tensor.matmul(
                                        pt[0:64 if single else 128,
                                           128 * slot : 128 * (slot + 1)],
                                        cpair[:, slot, 0, :] if single
                                        else cpair[:, slot, :, :],
                                        ident[:],
                                        is_transpose=True,
                                    )
                                psum_to_sbuf(
                                    stm[0:64, :] if single else stm[:],
                                    pt[0:64, :] if single else pt[:],
                                )
                                for nb in range(CPIX // 512):
                                    nc.tensor.matmul(
                                        pc[:, 512 * nb : 512 * (nb + 1)],
                                        wpt2[0:64 if single else 128,
                                             OC * pair : OC * (pair + 1)],
                                        stm[0:64 if single else 128,
                                            512 * nb : 512 * (nb + 1)],
                                        start=(pair == 0),
                                        stop=(pair == NPAIR - 1),
                                    )
                    if "nope" in abl:
                        nc.gpsimd.dma_start(
                            out[0:64, CPIX * ch : CPIX * ch + 1024],
                            g[0:64, :, :, :].rearrange("p a b c -> p (a b c)")[
                                :, 0:1024
                            ],
                        )
                    else:
                        osb = opool.tile([OC, CPIX], F32, name="osb")
                        nc.scalar.activation(
                            osb[:], pc[:], Act.Identity, bias=bias[:, 0:1]
                        )
                        nc.sync.dma_start(
                            out[:, CPIX * ch : CPIX * (ch + 1)], osb[:]
                        )

            pipelined = (
                "gonly" not in abl and "gonly2" not in abl
                and os.environ.get("PIPE", "1") == "1"
            )
            if pipelined:
                # software-pipeline stage B: a prologue computes the first
                # iteration's (weights, idxw); each loop body recomputes the
                # NEXT iteration's set (identical values, alternating bpool
                # buffers) at its head, so the current iteration's gathers —
                # the pacing stage — start immediately from ready buffers.
                pre = stage_a()
                if loop_n > 0:
                    if loop_n % 2 == 0 and os.environ.get("UNROLL", "2") == "2":
                        with tc.For_i(0, loop_n // 2, 1):
                            nxt = stage_a()
                            main_loops(pre[0], pre[1])
                            stage_a()
                            main_loops(nxt[0], nxt[1])
                    else:
                        with tc.For_i(0, loop_n, 1):
                            stage_a()
                            main_loops(pre[0], pre[1])
                else:
                    main_loops(pre[0], pre[1])
            elif loop_n > 0:
                if loop_n % 2 == 0 and os.environ.get("UNROLL", "2") == "2":
                    with tc.For_i(0, loop_n // 2, 1):
                        body()
                        body()
                else:
                    with tc.For_i(0, loop_n, 1):
                        body()
            else:
                body()
            if "gonly" in abl or "gonly2" in abl:
                dummy = cpool.tile([OC, 64], F32)
                nc.vector.memset(dummy[:], 1.0)
                nc.sync.dma_start(out[:, 0:64], dummy[:])

    nc.compile()
    return nc


def prep_core_inputs(x, offset, weight, bias, core):
    """Host-side shard/layout prep for one core. Pure layout, no math on
    tensor values (beyond the reference-mandated reshape semantics)."""
    s, half = core // 2, core % 2
    # interleaved row-pair NHWC: xpair[par*PIX + pr*128 + j] =
    #   [x[2pr+par, j, :], x[2pr+par+1, j, :]]
    xr = np.ascontiguousarray(x[s].transpose(1, 2, 0))          # [H, W, C]
    xpad = np.concatenate([xr, np.zeros((1, W, C), np.float32)], 0)  # [129,W,C]
    rows = np.stack([xpad[0:128], xpad[1:129]], 1)           # [128, 2, W, C]
    # xpair row (g0, j) = [x[g0, j, :], x[g0+1, j, :]]
    xpair = np.concatenate(
        [rows.transpose(0, 2, 1, 3).reshape(2 * PIX, 128),
         np.zeros((1, 128), np.float32)], 0)

    # offsets for this half, partition-major [128, SLOTS*18] (partition =
    # pixel%128 = image col, then (slot, channel) contiguous per partition)
    off = np.ascontiguousarray(
        offset[s, :, 64 * half : 64 * half + HALF, :]
        .transpose(1, 2, 0)          # [HALF, W, 18]
        .transpose(1, 0, 2)          # [W=128, HALF, 18]
        .reshape(128, SLOTS * 2 * KK)
    )
    # channel c=2t is x-offset, 2t+1 is y-offset (reference reshape
    # [kk,2,h,w]: x = off[:,:,0], y = off[:,:,1] -> channel t*2+0 / t*2+1)

    # per-tap weight slabs [128, 9*64]: rows sub*64+c (same W both rows,
    # for (row,ch)-contraction), cols t*64+o
    wfull = weight.reshape(OC, C, KK)
    wpair = np.zeros((128, KK * OC), np.float32)
    for t in range(KK):
        wt = wfull[:, :, t].T          # [c, o]
        wpair[0:64, t * OC : (t + 1) * OC] = wt
        wpair[64:128, t * OC : (t + 1) * OC] = wt
    # tap-pair slabs [128, 5*64]: rows = [tap 2p ch | tap 2p+1 ch] for
    # (tap-pair, ch)-contraction of fully-combined values; pair 4 = tap 8
    # alone in rows 0:64.
    wpair2 = np.zeros((128, NPAIR * OC), np.float32)
    for pr in range(NPAIR):
        wpair2[0:64, pr * OC : (pr + 1) * OC] = wfull[:, :, 2 * pr].T
        if 2 * pr + 1 < KK:
            wpair2[64:128, pr * OC : (pr + 1) * OC] = wfull[:, :, 2 * pr + 1].T

    yy = np.broadcast_to(
        (np.arange(SLOTS, dtype=np.float32) + 64 * half)[None, :], (128, SLOTS)
    ).copy()
    xxc = np.arange(128, dtype=np.float32).reshape(128, 1).copy()
    return {
        "xpair": np.ascontiguousarray(xpair).astype(np.float16),
        "offs": off.astype(np.float32),
        "wpair": wpair.astype(np.float16),
        "wpair2": wpair2.astype(np.float16),
        "yy": yy,
        "xx": xxc,
        "ident": np.eye(128, dtype=np.float16),
        "bias": bias.reshape(OC, 1).astype(np.float32),
    }


_CACHE = {}


def kernel(x, offset, weight, bias):
    x = np.asarray(x, np.float32)
    offset = np.asarray(offset, np.float32)
    weight = np.asarray(weight, np.float32)
    bias = np.asarray(bias, np.float32)
    if "nc" not in _CACHE:
        _CACHE["nc"] = build_program()
    nc = _CACHE["nc"]
    in_maps = [prep_core_inputs(x, offset, weight, bias, c) for c in range(8)]
    res = run_bass_kernel_spmd(nc, in_maps, core_ids=list(range(8)))
    outf = np.empty((B, OC, H, W), np.float32)
    for c in range(8):
        s, half = c // 2, c % 2
        outf[s, :, 64 * half : 64 * half + HALF, :] = res.results[c][
            "out"
        ].reshape(OC, HALF, W)
    return outf

